# revision 3
# baseline (speedup 1.0000x reference)
"""Trainium2 Bass kernel for a single-layer multi-head attention block.

Reference computation (per batch element):
    qkv = x @ w_qkv; q,k,v = split(qkv); 12 heads x 64
    out_h = softmax(q_h k_h^T / 8) v_h;  y = concat(out) @ w_out + b_out

Sharding: batch (8) data-parallel across 8 NeuronCores, one element/core.

Design notes (cost-model driven):
  - ACT exp runs at 1 elem/cycle/lane regardless of input dtype, so the v1
    DVE f32->bf16 "score cast" (~100us/iter of DVE time) is deleted: exp
    reads score PSUM f32 directly (PSUM access from ACT is even cheaper
    than SBUF: 172 vs 222 cycles).
  - PSUM is organized as small rotating tiles: a shared "big" pool
    (2 bufs x [128,1024] f32 = 4 banks) for qkv-proj / v-proj / scores /
    out-proj, and an "av" pool (2 bufs x [65,1024] = 4 banks). v1 used
    two single-buffered pools, serializing every phase on its drain.
  - The PE instruction stream is interleaved at ~1-3us granularity:
    scores of head h run with AV of head h-2 and one projection chunk, so
    the PE never parks on the ACT exp pipeline and holds its high p-state
    (2.4 GHz needs >3us of continuous busy; a parked PE drops to 1.2).
  - Per-head softmax denominators: ones-column trick in the AV matmul,
    reciprocal on DVE, partition-broadcast via DRAM bounce, multiply on
    DVE one head-window later (hides the bounce latency without blocking
    the in-order DVE stream).
  - DMA queues are specialized (x loads on sync, norm bounces on gpsimd,
    stores on the otherwise-idle ACT queue) so the next iteration's x
    load is never parked behind end-of-iteration DMAs; xT and the
    out-proj PSUM tiles are double-buffered/pool-separated so the next
    iteration's projections start during the epilogue drain.
  - BODY_UNROLL=2: two reps per For_i iteration butt-join in one basic
    block, halving the per-iteration staged-reset overhead (measured
    ~7% faster than unroll 1; reps=1 then executes 2 identical reps,
    which is output-identical).

Numerics: matmul operands bf16 (PSUM accumulation fp32), exp input fp32.
Measured rel err ~2e-3 vs the fp32 reference (gate 2e-2).
"""

import time

import numpy as np
import ml_dtypes

import concourse.bacc as bacc
import concourse.mybir as mybir
import concourse.tile as tile
from concourse.bass_utils import run_bass_kernel_spmd

N_CORES = 8
N = 1024          # tokens per batch element
E = 768           # embedding dim
H = 12            # heads
D = 64            # head dim
P = 128

f32 = mybir.dt.float32
bf16 = mybir.dt.bfloat16
AF = mybir.ActivationFunctionType
BF16 = ml_dtypes.bfloat16

BODY_UNROLL = 2


def build_nc(reps=1, abl=frozenset(), loop=True):
    nc = bacc.Bacc("TRN2", target_bir_lowering=False, debug=False,
                   num_devices=N_CORES)

    xT_d = nc.dram_tensor("xT", [E, N], bf16, kind="ExternalInput")
    w_qkv = nc.dram_tensor("w_qkv", [E, 3 * E], bf16, kind="ExternalInput")
    w_out = nc.dram_tensor("w_out", [E, E], bf16, kind="ExternalInput")
    b_out = nc.dram_tensor("b_out", [E], f32, kind="ExternalInput")
    ones_c = nc.dram_tensor("ones_const", [1], bf16, kind="ExternalInput")
    out = nc.dram_tensor("out", [N, E], f32, kind="ExternalOutput")
    inv_scratch = nc.dram_tensor("inv_scratch", [H, N], f32)

    with tile.TileContext(nc) as tc:
      with tc.tile_pool(name="pers", bufs=1) as pers, \
           tc.tile_pool(name="bigp", bufs=2, space="PSUM") as bigp, \
           tc.tile_pool(name="avp", bufs=2, space="PSUM") as avp, \
           tc.tile_pool(name="wq", bufs=1) as wq_pool, \
           tc.tile_pool(name="xp", bufs=2) as xp, \
           tc.tile_pool(name="wout", bufs=1) as wout_pool:

        qkT = pers.tile([P, 12, N], bf16, tag="qkT")   # feat-major q|k
        v_aug = pers.tile([P, 8, H, D + 1], bf16, tag="v_aug")
        outT = pers.tile([P, 6, N], bf16, tag="outT")
        b_bc = pers.tile([P, E], f32, tag="b_bc")

        # ---- loop-invariant loads: weights, bias, ones column ----
        nc.sync.dma_start(out=b_bc[:], in_=b_out[None, :].to_broadcast((P, E)))
        nc.sync.dma_start(
            out=v_aug[:].rearrange("p a h d -> p (a h) d")[:, :, D:D + 1],
            in_=ones_c[None, None, :].to_broadcast((P, 8 * H, 1)))
        wqs = []
        for kc in range(6):
            wq = wq_pool.tile([P, 3 * E], bf16, tag=f"wq{kc}", name=f"wq_{kc}")
            nc.sync.dma_start(out=wq[:], in_=w_qkv[kc * P:(kc + 1) * P, :])
            wqs.append(wq)
        wos = []
        for fc in range(6):
            wo = wout_pool.tile([P, E], bf16, tag=f"wo{fc}", name=f"wo_{fc}")
            nc.sync.dma_start(out=wo[:], in_=w_out[fc * P:(fc + 1) * P, :])
            wos.append(wo)

        out_v = out.rearrange("(t p) e -> p t e", p=P)

        def emit_body(it=0):
            # ---- x load: contiguous bf16 (pre-transposed on host),
            #      alone on the sync queue so next-iter loads are never
            #      parked behind the norm-bounce DMAs ----
            xT = xp.tile([P, 6, N], bf16, tag="xT", name=f"xT_{it}")
            for ec in range(6):
                nc.sync.dma_start(
                    out=xT[:, ec, :], in_=xT_d[ec * P:(ec + 1) * P, :])

            with tc.tile_pool(name="expp", bufs=26) as expp, \
                 tc.tile_pool(name="invp", bufs=3) as invp, \
                 tc.tile_pool(name="ibcp", bufs=3) as ibcp, \
                 tc.tile_pool(name="fsp", bufs=2) as fsp:

                def emit_j(j):
                    # one 128-col chunk of the q|k projection
                    pq = bigp.tile([P, N], f32, tag="big",
                                   name=f"pq_{it}_{j}")
                    for kc in range(6):       # kc outer: nt pair shares lhsT
                        for nt in range(2):
                            nc.tensor.matmul(
                                pq[:, nt * 512:(nt + 1) * 512],
                                wqs[kc][:, j * P:(j + 1) * P],
                                xT[:, kc, nt * 512:(nt + 1) * 512],
                                start=(kc == 0), stop=(kc == 5))
                    nc.vector.tensor_copy(qkT[:, j, :], pq[:])

                def emit_v(t):
                    # one 128-token tile of the v projection (token-major)
                    pv = bigp.tile([P, N], f32, tag="big",
                                   name=f"pv_{it}_{t}")
                    for kc in range(6):
                        for vf, f0, fw in ((0, 0, 512), (1, 512, 256)):
                            nc.tensor.matmul(
                                pv[:, f0:f0 + fw],
                                xT[:, kc, t * P:(t + 1) * P],
                                wqs[kc][:, 2 * E + f0:2 * E + f0 + fw],
                                start=(kc == 0), stop=(kc == 5))
                    nc.vector.tensor_copy(
                        v_aug[:, t, :, 0:D],
                        pv[:, 0:E].rearrange("p (h d) -> p h d", d=D))

                ets = {h: [] for h in range(H)}

                def emit_sc(h, m):
                    # scores + exp for one 128-key tile of head h
                    qp = (h % 2) * D
                    jq = h // 2
                    jk = 6 + jq
                    ps = bigp.tile([P, N], f32, tag="big",
                                   name=f"sc_{it}_{h}_{m}")
                    for nt in range(2):
                        nc.tensor.matmul(
                            ps[:, nt * 512:(nt + 1) * 512],
                            qkT[qp:qp + D, jk, m * P:(m + 1) * P],
                            qkT[qp:qp + D, jq, nt * 512:(nt + 1) * 512],
                            start=True, stop=True)
                    et = expp.tile([P, N], bf16, tag="expp",
                                   name=f"et_{it}_{h}_{m}")
                    nc.scalar.activation(et[:], ps[:], AF.Exp, scale=0.125)
                    ets[h].append(et)

                def emit_av(h, m, pav):
                    for nt in range(2):
                        nc.tensor.matmul(
                            pav[0:D + 1, nt * 512:(nt + 1) * 512],
                            v_aug[:, m, h, :],
                            ets[h][m][:, nt * 512:(nt + 1) * 512],
                            start=(m == 0), stop=(m == 7))

                def emit_norm1(h, pav):
                    # reciprocal of the ones-row sum + DRAM-bounce broadcast
                    inv = invp.tile([D + 1, N], f32, tag="invp",
                                    name=f"inv_{it}_{h}")
                    nc.vector.reciprocal(inv[D:D + 1, :], pav[D:D + 1, :])
                    nc.gpsimd.dma_start(out=inv_scratch[h][None, :],
                                          in_=inv[D:D + 1, :])
                    ibc = ibcp.tile([D, N], f32, tag="ibcp",
                                    name=f"ibc_{it}_{h}")
                    nc.gpsimd.dma_start(
                        out=ibc[:],
                        in_=inv_scratch[h][None, :].to_broadcast((D, N)))
                    return ibc

                def emit_norm2(h, pav, ibc):
                    qp = (h % 2) * D
                    jq = h // 2
                    nc.vector.tensor_mul(outT[qp:qp + D, jq, :],
                                         pav[0:D, :], ibc[:])

                # ---- prologue: q|k chunks for heads 0..3 ----
                for j in (0, 6, 1, 7):
                    emit_j(j)

                # ---- interleaved attention pipeline ----
                jq2 = (2, 8, 3, 9, 4, 10, 5, 11)
                pavs = {}
                ibcs = {}
                for h in range(H):
                    hav = h - 2
                    if hav >= 0:
                        pavs[hav] = avp.tile([D + 1, N], f32, tag="avp",
                                             name=f"pav_{it}_{hav}")
                    for m in range(8):
                        emit_sc(h, m)
                        if hav >= 0:
                            emit_av(hav, m, pavs[hav])
                        if h in (0, 1) and m % 2 == 1:
                            emit_v((h * 8 + m) // 2)
                        if 2 <= h <= 5 and m in (3, 7):
                            emit_j(jq2[(h - 2) * 2 + (1 if m == 7 else 0)])
                    if hav >= 0:
                        ibcs[hav] = emit_norm1(hav, pavs[hav])
                    if hav - 1 >= 0:
                        emit_norm2(hav - 1, pavs[hav - 1], ibcs[hav - 1])
                        del pavs[hav - 1]

                # ---- epilogue: last two AV heads ----
                for hav in (10, 11):
                    pavs[hav] = avp.tile([D + 1, N], f32, tag="avp",
                                         name=f"pav_{it}_{hav}")
                    for m in range(8):
                        emit_av(hav, m, pavs[hav])
                    ibcs[hav] = emit_norm1(hav, pavs[hav])
                    emit_norm2(hav - 1, pavs[hav - 1], ibcs[hav - 1])
                    del pavs[hav - 1]
                emit_norm2(11, pavs[11], ibcs[11])
                del pavs[11]

                # ---- output projection + bias + store ----
                # fc=5 reads outT heads 10/11 whose normalize lands last;
                # for the first two tiles it is deferred so the PE has
                # fc0-4 of both in flight while the last bounce completes
                def emit_out_mm(pf, t, fcs, start, stop):
                    for fc in fcs:        # fc outer: ft pair shares lhsT
                        for ft, f0, fw in ((0, 0, 512), (1, 512, 256)):
                            nc.tensor.matmul(
                                pf[:, f0:f0 + fw],
                                outT[:, fc, t * P:(t + 1) * P],
                                wos[fc][:, f0:f0 + fw],
                                start=(fc == fcs[0] and start),
                                stop=(fc == fcs[-1] and stop))

                def emit_out_fin(pf, t):
                    fs = fsp.tile([P, E], f32, tag="fsp",
                                  name=f"fs_{it}_{t}")
                    nc.vector.tensor_add(fs[:], pf[:, 0:E], b_bc[:])
                    # stores go on the ACT queue: ACT's exp work is done by
                    # now, and this keeps the sync queue free so the next
                    # iteration's x load isn't parked behind the stores
                    nc.scalar.dma_start(out=out_v[:, t, :], in_=fs[:])

                # pf tiles live in the AV pool so bigp stays free for the
                # next iteration's projections while the epilogue drains
                pf0 = avp.tile([P, N], f32, tag="avp", name=f"pf_{it}_0")
                pf1 = avp.tile([P, N], f32, tag="avp", name=f"pf_{it}_1")
                emit_out_mm(pf0, 0, (0, 1, 2, 3, 4), True, False)
                emit_out_mm(pf1, 1, (0, 1, 2, 3, 4), True, False)
                emit_out_mm(pf0, 0, (5,), False, True)
                emit_out_fin(pf0, 0)
                emit_out_mm(pf1, 1, (5,), False, True)
                emit_out_fin(pf1, 1)
                for t in range(2, 8):
                    pf = avp.tile([P, N], f32, tag="avp",
                                  name=f"pf_{it}_{t}")
                    emit_out_mm(pf, t, (0, 1, 2, 3, 4, 5), True, True)
                    emit_out_fin(pf, t)

        if loop:
            n_iter = max(1, reps // BODY_UNROLL)
            with tc.For_i(0, n_iter, 1, staggered_reset=True,
                          hint_engines=tuple(mybir.ALL_ENGINES)):
                for _u in range(BODY_UNROLL):
                    emit_body(it=_u)
        else:
            for _u in range(reps):
                emit_body(it=_u)

    nc.compile()
    return nc


_NC = None


def _get_nc():
    global _NC
    if _NC is None:
        _NC = build_nc()
    return _NC


def make_in_maps(x, w_qkv, w_out, b_out):
    """Host-side input marshalling: per-core transposed bf16 x + shared
    bf16 weights."""
    x = np.asarray(x)
    wq16 = np.ascontiguousarray(np.asarray(w_qkv, dtype=np.float32)
                                .astype(BF16))
    wo16 = np.ascontiguousarray(np.asarray(w_out, dtype=np.float32)
                                .astype(BF16))
    b_out = np.ascontiguousarray(np.asarray(b_out, dtype=np.float32))
    one = np.ones(1, dtype=BF16)
    return [
        {"xT": np.ascontiguousarray(
             np.asarray(x[i], dtype=np.float32).T.astype(BF16)),
         "w_qkv": wq16, "w_out": wo16, "b_out": b_out, "ones_const": one}
        for i in range(N_CORES)
    ]


def kernel(x, w_qkv, w_out, b_out):
    nc = _get_nc()
    in_maps = make_in_maps(x, w_qkv, w_out, b_out)
    last_exc = None
    for attempt in range(4):   # retry transient device errors
        try:
            res = run_bass_kernel_spmd(nc, in_maps,
                                       core_ids=list(range(N_CORES)))
            return np.stack([res.results[i]["out"] for i in range(N_CORES)],
                            axis=0)
        except Exception as e:   # noqa: BLE001
            last_exc = e
            time.sleep(2.0 * (attempt + 1))
    raise last_exc


# revision 4
# speedup vs baseline: 1.0306x; 1.0306x over previous
"""Trainium2 Bass kernel for a single-layer multi-head attention block.

Reference computation (per batch element):
    qkv = x @ w_qkv; q,k,v = split(qkv); 12 heads x 64
    out_h = softmax(q_h k_h^T / 8) v_h;  y = concat(out) @ w_out + b_out

Sharding: batch (8) data-parallel across 8 NeuronCores, one element/core.

Design notes (cost-model driven):
  - ACT exp runs at 1 elem/cycle/lane regardless of input dtype, so the v1
    DVE f32->bf16 "score cast" (~100us/iter of DVE time) is deleted: exp
    reads score PSUM f32 directly (PSUM access from ACT is even cheaper
    than SBUF: 172 vs 222 cycles).
  - PSUM is organized as small rotating tiles: a shared "big" pool
    (2 bufs x [128,1024] f32 = 4 banks) for qkv-proj / v-proj / scores /
    out-proj, and an "av" pool (2 bufs x [65,1024] = 4 banks). v1 used
    two single-buffered pools, serializing every phase on its drain.
  - The PE instruction stream is interleaved at ~1-3us granularity:
    scores of head h run with AV of head h-2 and one projection chunk, so
    the PE never parks on the ACT exp pipeline and holds its high p-state
    (2.4 GHz needs >3us of continuous busy; a parked PE drops to 1.2).
  - Per-head softmax denominators: ones-column trick in the AV matmul,
    reciprocal on DVE, partition-broadcast via DRAM bounce, multiply on
    DVE one head-window later (hides the bounce latency without blocking
    the in-order DVE stream).

Numerics: matmul operands bf16 (PSUM accumulation fp32), exp input fp32.
Measured rel err ~2e-3 vs the fp32 reference (gate 2e-2).
"""

import time

import numpy as np
import ml_dtypes

import concourse.bacc as bacc
import concourse.mybir as mybir
import concourse.tile as tile
from concourse.bass_utils import run_bass_kernel_spmd

N_CORES = 8
N = 1024          # tokens per batch element
E = 768           # embedding dim
H = 12            # heads
D = 64            # head dim
P = 128

f32 = mybir.dt.float32
bf16 = mybir.dt.bfloat16
AF = mybir.ActivationFunctionType
BF16 = ml_dtypes.bfloat16

BODY_UNROLL = 4


def build_nc(reps=1, abl=frozenset(), loop=True):
    nc = bacc.Bacc("TRN2", target_bir_lowering=False, debug=False,
                   num_devices=N_CORES)

    xT_d = nc.dram_tensor("xT", [E, N], bf16, kind="ExternalInput")
    w_qkv = nc.dram_tensor("w_qkv", [E, 3 * E], bf16, kind="ExternalInput")
    w_out = nc.dram_tensor("w_out", [E, E], bf16, kind="ExternalInput")
    b_out = nc.dram_tensor("b_out", [E], f32, kind="ExternalInput")
    ones_c = nc.dram_tensor("ones_const", [1], bf16, kind="ExternalInput")
    out = nc.dram_tensor("out", [N, E], f32, kind="ExternalOutput")
    inv_scratch = nc.dram_tensor("inv_scratch", [H, N], f32)

    with tile.TileContext(nc) as tc:
      with tc.tile_pool(name="pers", bufs=1) as pers, \
           tc.tile_pool(name="bigp", bufs=2, space="PSUM") as bigp, \
           tc.tile_pool(name="avp", bufs=2, space="PSUM") as avp, \
           tc.tile_pool(name="wq", bufs=1) as wq_pool, \
           tc.tile_pool(name="xp", bufs=2) as xp, \
           tc.tile_pool(name="wout", bufs=1) as wout_pool:

        qkT = pers.tile([P, 12, N], bf16, tag="qkT")   # feat-major q|k
        v_aug = pers.tile([P, 8, H, D + 1], bf16, tag="v_aug")
        outT = pers.tile([P, 6, N], bf16, tag="outT")
        b_bc = pers.tile([P, E], f32, tag="b_bc")

        # ---- loop-invariant loads: weights, bias, ones column ----
        nc.sync.dma_start(out=b_bc[:], in_=b_out[None, :].to_broadcast((P, E)))
        nc.sync.dma_start(
            out=v_aug[:].rearrange("p a h d -> p (a h) d")[:, :, D:D + 1],
            in_=ones_c[None, None, :].to_broadcast((P, 8 * H, 1)))
        wqs = []
        for kc in range(6):
            wq = wq_pool.tile([P, 3 * E], bf16, tag=f"wq{kc}", name=f"wq_{kc}")
            nc.sync.dma_start(out=wq[:], in_=w_qkv[kc * P:(kc + 1) * P, :])
            wqs.append(wq)
        wos = []
        for fc in range(6):
            wo = wout_pool.tile([P, E], bf16, tag=f"wo{fc}", name=f"wo_{fc}")
            nc.sync.dma_start(out=wo[:], in_=w_out[fc * P:(fc + 1) * P, :])
            wos.append(wo)

        out_v = out.rearrange("(t p) e -> p t e", p=P)

        def emit_body(it=0):
            # ---- x load: one DMA for all 1.5MB (fewer queue round
            #      trips); alone on the sync queue so next-iter loads are
            #      never parked behind the norm-bounce DMAs ----
            xT = xp.tile([P, 6, N], bf16, tag="xT", name=f"xT_{it}")
            nc.sync.dma_start(
                out=xT[:], in_=xT_d.rearrange("(c p) n -> p c n", p=P))

            with tc.tile_pool(name="expp", bufs=26) as expp, \
                 tc.tile_pool(name="invp", bufs=3) as invp, \
                 tc.tile_pool(name="ibcp", bufs=3) as ibcp, \
                 tc.tile_pool(name="fsp", bufs=2) as fsp:

                def emit_j(j):
                    # one 128-col chunk of the q|k projection
                    pq = bigp.tile([P, N], f32, tag="big",
                                   name=f"pq_{it}_{j}")
                    for kc in range(6):       # kc outer: nt pair shares lhsT
                        for nt in range(2):
                            nc.tensor.matmul(
                                pq[:, nt * 512:(nt + 1) * 512],
                                wqs[kc][:, j * P:(j + 1) * P],
                                xT[:, kc, nt * 512:(nt + 1) * 512],
                                start=(kc == 0), stop=(kc == 5))
                    nc.vector.tensor_copy(qkT[:, j, :], pq[:])

                def emit_v(t):
                    # one 128-token tile of the v projection (token-major)
                    pv = bigp.tile([P, N], f32, tag="big",
                                   name=f"pv_{it}_{t}")
                    for kc in range(6):
                        for vf, f0, fw in ((0, 0, 512), (1, 512, 256)):
                            nc.tensor.matmul(
                                pv[:, f0:f0 + fw],
                                xT[:, kc, t * P:(t + 1) * P],
                                wqs[kc][:, 2 * E + f0:2 * E + f0 + fw],
                                start=(kc == 0), stop=(kc == 5))
                    nc.vector.tensor_copy(
                        v_aug[:, t, :, 0:D],
                        pv[:, 0:E].rearrange("p (h d) -> p h d", d=D))

                ets = {h: [] for h in range(H)}

                def emit_sc(h, m):
                    # scores + exp for one 128-key tile of head h
                    qp = (h % 2) * D
                    jq = h // 2
                    jk = 6 + jq
                    ps = bigp.tile([P, N], f32, tag="big",
                                   name=f"sc_{it}_{h}_{m}")
                    for nt in range(2):
                        nc.tensor.matmul(
                            ps[:, nt * 512:(nt + 1) * 512],
                            qkT[qp:qp + D, jk, m * P:(m + 1) * P],
                            qkT[qp:qp + D, jq, nt * 512:(nt + 1) * 512],
                            start=True, stop=True)
                    et = expp.tile([P, N], bf16, tag="expp",
                                   name=f"et_{it}_{h}_{m}")
                    nc.scalar.activation(et[:], ps[:], AF.Exp, scale=0.125)
                    ets[h].append(et)

                def emit_av(h, m, pav):
                    for nt in range(2):
                        nc.tensor.matmul(
                            pav[0:D + 1, nt * 512:(nt + 1) * 512],
                            v_aug[:, m, h, :],
                            ets[h][m][:, nt * 512:(nt + 1) * 512],
                            start=(m == 0), stop=(m == 7))

                def emit_norm1(h, pav):
                    # reciprocal of the ones-row sum + DRAM-bounce broadcast
                    inv = invp.tile([D + 1, N], f32, tag="invp",
                                    name=f"inv_{it}_{h}")
                    nc.vector.reciprocal(inv[D:D + 1, :], pav[D:D + 1, :])
                    nc.gpsimd.dma_start(out=inv_scratch[h][None, :],
                                          in_=inv[D:D + 1, :])
                    ibc = ibcp.tile([D, N], f32, tag="ibcp",
                                    name=f"ibc_{it}_{h}")
                    nc.gpsimd.dma_start(
                        out=ibc[:],
                        in_=inv_scratch[h][None, :].to_broadcast((D, N)))
                    return ibc

                def emit_norm2(h, pav, ibc):
                    qp = (h % 2) * D
                    jq = h // 2
                    nc.vector.tensor_mul(outT[qp:qp + D, jq, :],
                                         pav[0:D, :], ibc[:])

                # ---- prologue: q|k chunks for heads 0..3 ----
                for j in (0, 6, 1, 7):
                    emit_j(j)

                # ---- interleaved attention pipeline ----
                jq2 = (2, 8, 3, 9, 4, 10, 5, 11)
                pavs = {}
                ibcs = {}
                for h in range(H):
                    hav = h - 2
                    if hav >= 0:
                        pavs[hav] = avp.tile([D + 1, N], f32, tag="avp",
                                             name=f"pav_{it}_{hav}")
                    for m in range(8):
                        emit_sc(h, m)
                        if hav >= 0:
                            emit_av(hav, m, pavs[hav])
                        if h in (0, 1) and m % 2 == 1:
                            emit_v((h * 8 + m) // 2)
                        if 2 <= h <= 5 and m in (3, 7):
                            emit_j(jq2[(h - 2) * 2 + (1 if m == 7 else 0)])
                    if hav >= 0:
                        ibcs[hav] = emit_norm1(hav, pavs[hav])
                    if hav - 1 >= 0:
                        emit_norm2(hav - 1, pavs[hav - 1], ibcs[hav - 1])
                        del pavs[hav - 1]

                # ---- epilogue: last two AV heads ----
                for hav in (10, 11):
                    pavs[hav] = avp.tile([D + 1, N], f32, tag="avp",
                                         name=f"pav_{it}_{hav}")
                    for m in range(8):
                        emit_av(hav, m, pavs[hav])
                    ibcs[hav] = emit_norm1(hav, pavs[hav])
                    emit_norm2(hav - 1, pavs[hav - 1], ibcs[hav - 1])
                    del pavs[hav - 1]
                emit_norm2(11, pavs[11], ibcs[11])
                del pavs[11]

                # ---- output projection + bias + store ----
                # fc=5 reads outT heads 10/11 whose normalize lands last;
                # for the first two tiles it is deferred so the PE has
                # fc0-4 of both in flight while the last bounce completes
                def emit_out_mm(pf, t, fcs, start, stop):
                    for fc in fcs:        # fc outer: ft pair shares lhsT
                        for ft, f0, fw in ((0, 0, 512), (1, 512, 256)):
                            nc.tensor.matmul(
                                pf[:, f0:f0 + fw],
                                outT[:, fc, t * P:(t + 1) * P],
                                wos[fc][:, f0:f0 + fw],
                                start=(fc == fcs[0] and start),
                                stop=(fc == fcs[-1] and stop))

                def emit_out_fin(pf, t):
                    fs = fsp.tile([P, E], f32, tag="fsp",
                                  name=f"fs_{it}_{t}")
                    nc.vector.tensor_add(fs[:], pf[:, 0:E], b_bc[:])
                    # stores go on the ACT queue: ACT's exp work is done by
                    # now, and this keeps the sync queue free so the next
                    # iteration's x load isn't parked behind the stores
                    nc.scalar.dma_start(out=out_v[:, t, :], in_=fs[:])

                # pf tiles live in the AV pool so bigp stays free for the
                # next iteration's projections while the epilogue drains
                pf0 = avp.tile([P, N], f32, tag="avp", name=f"pf_{it}_0")
                pf1 = avp.tile([P, N], f32, tag="avp", name=f"pf_{it}_1")
                emit_out_mm(pf0, 0, (0, 1, 2, 3, 4), True, False)
                emit_out_mm(pf1, 1, (0, 1, 2, 3, 4), True, False)
                emit_out_mm(pf0, 0, (5,), False, True)
                emit_out_fin(pf0, 0)
                emit_out_mm(pf1, 1, (5,), False, True)
                emit_out_fin(pf1, 1)
                for t in range(2, 8):
                    pf = avp.tile([P, N], f32, tag="avp",
                                  name=f"pf_{it}_{t}")
                    emit_out_mm(pf, t, (0, 1, 2, 3, 4, 5), True, True)
                    emit_out_fin(pf, t)

        if loop:
            n_iter = max(1, reps // BODY_UNROLL)
            with tc.For_i(0, n_iter, 1, staggered_reset=True,
                          hint_engines=tuple(mybir.ALL_ENGINES)):
                for _u in range(BODY_UNROLL):
                    emit_body(it=_u)
        else:
            for _u in range(reps):
                emit_body(it=_u)

    nc.compile()
    return nc


_NC = None


def _get_nc():
    global _NC
    if _NC is None:
        _NC = build_nc()
    return _NC


def make_in_maps(x, w_qkv, w_out, b_out):
    """Host-side input marshalling: per-core transposed bf16 x + shared
    bf16 weights."""
    x = np.asarray(x)
    wq16 = np.ascontiguousarray(np.asarray(w_qkv, dtype=np.float32)
                                .astype(BF16))
    wo16 = np.ascontiguousarray(np.asarray(w_out, dtype=np.float32)
                                .astype(BF16))
    b_out = np.ascontiguousarray(np.asarray(b_out, dtype=np.float32))
    one = np.ones(1, dtype=BF16)
    return [
        {"xT": np.ascontiguousarray(
             np.asarray(x[i], dtype=np.float32).T.astype(BF16)),
         "w_qkv": wq16, "w_out": wo16, "b_out": b_out, "ones_const": one}
        for i in range(N_CORES)
    ]


def kernel(x, w_qkv, w_out, b_out):
    nc = _get_nc()
    in_maps = make_in_maps(x, w_qkv, w_out, b_out)
    last_exc = None
    for attempt in range(4):   # retry transient device errors
        try:
            res = run_bass_kernel_spmd(nc, in_maps,
                                       core_ids=list(range(N_CORES)))
            return np.stack([res.results[i]["out"] for i in range(N_CORES)],
                            axis=0)
        except Exception as e:   # noqa: BLE001
            last_exc = e
            time.sleep(2.0 * (attempt + 1))
    raise last_exc
